# revision 34
# baseline (speedup 1.0000x reference)
"""Trainium2 Bass kernel for nn_ActionEncoder (moe_routing).

Math (derived from the reference):
  For sample b with t = action_types[b], i0, i1 = action_indecies[b]:
    type 0: out = tanh(W0[:, i0] + b0)
    type 1: out = tanh(W1[:, i0] + W1[:, 64 + i1] + b1)

Routing: the host stable-sorts each core's 65536 samples by type (pure
permutation; inverse-applied to the output).  Device columns then hold
same-type data with unshifted 6-bit keys, so ONE is_equal builds the
one-hot marks for a whole quad:
  - type-0 block: each 512-wide column group packs TWO samples per column
    (rows 0-63 mark i0 of sample A, rows 64-127 mark i0 of sample B); a
    block-diagonal [128, 8] table gathers both samples' 4 features.
  - type-1 block: one sample per column (rows 0-63 mark i0, 64-127 mark
    64+i1); a [128, 4] table (W1.T + b1/2 for both halves) gathers z.
Pipeline per quad (4 groups of 512 cols):
  4 broadcast matmuls spread keys into psum -> 2 psum->sbuf fp16 copies
  (rotating ACT/DVE) -> 1 DVE is_equal (4x mode) -> 4 gather matmuls
  (concurrent col groups) accumulating several quads into one psum bank
  via column-shifted table variants -> one tanh per bank -> fp16 out DMA.
Host reassembles/unsorts to [B, 4] fp32.
"""

import os

import numpy as np

N_CORES = 8
P = 128
S = 512
DVE_COPY_MOD = int(os.environ.get("ACTENC_DVE_COPY_MOD", "3"))

# column-block geometry (per core): type-0 block packs 2 samples/col
G0 = 36                  # type-0 groups: capacity 2*36*512 = 36864 samples
G1 = 68                  # type-1 groups: capacity 68*512 = 34816 samples
G = G0 + G1              # 104 groups = 26 quads
GH = G // 2              # 52 group-pairs per IP half
NQ0, NQ1 = G0 // 4, G1 // 4          # 9 t0 quads, 17 t1 quads
# psum bank packing: t0 banks hold 4 quads (8 rows/group), t1 banks 8 (4 rows)
BANKS0 = [4, 4, 1]       # quads per t0 bank
BANKS1 = [8, 8, 1]       # quads per t1 bank

_NC_CACHE = {}


def _build_nc(b_cols):
    import concourse.mybir as mybir
    from concourse import bacc
    from concourse.tile import TileContext

    f32 = mybir.dt.float32
    i32 = mybir.dt.int32
    f16 = mybir.dt.float16
    eq = mybir.AluOpType.is_equal

    assert b_cols == G * S

    nc = bacc.Bacc("TRN2", target_bir_lowering=False, debug=False)
    idx = nc.dram_tensor("idx", [b_cols, 2], i32, kind="ExternalInput")
    # t0 table: 4 col-shifted block-diag variants [128, 32] at cols 32q
    taba = nc.dram_tensor("taba", [P, 128], f16, kind="ExternalInput")
    # t1 table: 8 col-shifted variants [128, 32] at cols 32q
    tabb = nc.dram_tensor("tabb", [P, 256], f16, kind="ExternalInput")
    selq = nc.dram_tensor("selq", [P, 128 * GH], f16, kind="ExternalInput")
    outa = nc.dram_tensor("outa", [len(BANKS0), 4, 32, S], f16,
                          kind="ExternalOutput")
    outb = nc.dram_tensor("outb", [len(BANKS1), 4, 32, S], f16,
                          kind="ExternalOutput")

    # pair-contiguous DRAM view: [2 halves, GH groups, 2*S] (4KB rows)
    idxp = idx.rearrange("(h g s) c -> h g (s c)", h=2, s=S)

    with TileContext(nc) as tc:
        with tc.tile_pool(name="const", bufs=1) as cpool, \
             tc.tile_pool(name="oh", bufs=6) as ohpool, \
             tc.tile_pool(name="raws", bufs=4) as rpool, \
             tc.tile_pool(name="stage", bufs=2) as spool, \
             tc.tile_pool(name="psb", bufs=3, space="PSUM") as pbpool, \
             tc.tile_pool(name="pszp", bufs=2, space="PSUM") as pzpool:

            # ---- constants ----
            # selector band (host-shipped): view Q[:, 128j : 128j+128]
            # = [e_j x64 | e_{64+j} x64]
            NQb = 128 * GH
            Q = cpool.tile([P, NQb], f16, tag="Q")
            for qc in range(4):
                csl = slice(qc * NQb // 4, (qc + 1) * NQb // 4)
                nc.sync.dma_start(out=Q[:, csl], in_=selq[:, csl])

            # iota2[d] = d mod 64
            ic = cpool.tile([P, 1], i32, tag="ic")
            nc.gpsimd.iota(ic[0:64, :], pattern=[[1, 1]], base=0,
                           channel_multiplier=1)
            nc.gpsimd.iota(ic[64:128, :], pattern=[[1, 1]], base=0,
                           channel_multiplier=1)
            iota2 = cpool.tile([P, 1], f32, tag="iota2")
            nc.vector.tensor_single_scalar(iota2[:], ic[:], 0.0,
                                           mybir.AluOpType.add)

            TA = cpool.tile([P, 128], f16, tag="TA")
            TB = cpool.tile([P, 256], f16, tag="TB")
            nc.sync.dma_start(out=TA[:], in_=taba[:])
            nc.sync.dma_start(out=TB[:], in_=tabb[:])

            # ---- load index pairs; IP[h] rows 0..GH-1 = slot0 keys of the
            #      half's groups, rows 64..64+GH-1 = slot1 keys. Chunked so
            #      early quads start before all input has landed. ----
            IPraw = [cpool.tile([P, 2 * S], i32, tag=f"IPraw{h}",
                                name=f"IPraw{h}") for h in range(2)]
            IP = [cpool.tile([P, S], f16, tag=f"IP{h}", name=f"IP{h}")
                  for h in range(2)]
            for h in range(2):
                # rows GH..63 / 64+GH..127 are never selected but stream
                # through the PE with weight 0 -- zero the whole tile first
                # so stale NaN bit patterns cannot poison psum (0*NaN=NaN)
                nc.vector.memset(IP[h][:], 0)
            # chunks must start 32-partition-aligned
            CHUNKS = [(0, 32), (32, GH - 32)]
            for h in range(2):
                prw3 = IPraw[h][:].rearrange("p (s c) -> p c s", c=2)
                for off, cnt in CHUNKS:
                    gsl = slice(off, off + cnt)
                    for c in range(2):
                        rsl = slice(64 * c + off, 64 * c + off + cnt)
                        nc.sync.dma_start(out=IPraw[h][rsl, :],
                                          in_=idxp[h, gsl])
                        nc.vector.tensor_copy(out=IP[h][rsl, :],
                                              in_=prw3[rsl, c, :])

            # ---- quad schedule: (is_t1, bank, qq(pos in bank), start, stop)
            sched = []
            qt = 0
            for bi, nq in enumerate(BANKS0):
                for qq in range(nq):
                    sched.append((0, bi, qq, qq == 0, qq == nq - 1))
            for bi, nq in enumerate(BANKS1):
                for qq in range(nq):
                    sched.append((1, bi, qq, qq == 0, qq == nq - 1))
            NPI = len(sched)
            assert NPI == NQ0 + NQ1

            def emit_spread(pi):
                raw = rpool.tile([P, 4 * S], f16, tag="raw", name="raw")
                for half in range(2):
                    psb = pbpool.tile([P, 2 * S], f32, tag="psb", name="psb")
                    for i in range(2):
                        g = pi * 4 + half * 2 + i
                        h, j = divmod(g, GH)
                        sel = Q[:, 128 * j:128 * j + 128]
                        nc.tensor.matmul(psb[:, i * S:(i + 1) * S],
                                         lhsT=sel, rhs=IP[h][:],
                                         start=True, stop=True)
                    rsl = slice(half * 2 * S, half * 2 * S + 2 * S)
                    ci = pi * 2 + half
                    if ci % DVE_COPY_MOD == DVE_COPY_MOD - 1:
                        nc.vector.tensor_copy(out=raw[:, rsl], in_=psb[:])
                    else:
                        nc.scalar.copy(raw[:, rsl], psb[:])
                return raw

            psz = None
            raw_next = emit_spread(0)
            for pi in range(NPI):
                is_t1, bi, qq, st, sp = sched[pi]
                if st:
                    psz = pzpool.tile([P, S], f32, tag="psz")
                raw = raw_next
                oh = ohpool.tile([P, 4 * S], f16, tag="oh", name="oh")
                nc.vector.tensor_single_scalar(oh[:], raw[:], iota2[:], eq)
                if pi + 1 < NPI:
                    raw_next = emit_spread(pi + 1)
                tab = TB if is_t1 else TA
                for a in range(4):
                    pz = psz[32 * a:32 * a + 32, :]
                    nc.tensor.matmul(
                        pz, lhsT=tab[:, 32 * qq:32 * qq + 32],
                        rhs=oh[:, a * S:a * S + S],
                        start=st, stop=sp,
                        tile_position=(0, 32 * a),
                        skip_group_check=True)
                if sp:
                    stage = spool.tile([P, S], f16, tag="stage")
                    nc.scalar.activation(
                        out=stage[:], in_=psz[:],
                        func=mybir.ActivationFunctionType.Tanh)
                    outx = outb if is_t1 else outa
                    for a in range(4):
                        nc.sync.dma_start(out=outx[bi, a],
                                          in_=stage[32 * a:32 * a + 32, :])

    nc.compile()
    return nc


def _selq():
    # col block j (j < GH): rows 0-63 = e_j, rows 64-127 = e_{64+j}
    Qm = np.zeros((128, 128 * GH), np.float16)
    j = np.arange(GH)
    for r in range(64):
        Qm[j, 128 * j + r] = 1
        Qm[64 + j, 128 * j + 64 + r] = 1
    return Qm


def _tables(W0, b0, W1, b1):
    W0 = np.asarray(W0, np.float32)
    W1 = np.asarray(W1, np.float32)
    b0 = np.asarray(b0, np.float32).reshape(-1)
    b1 = np.asarray(b1, np.float32).reshape(-1)
    T0f = (W0.T + b0).astype(np.float16)          # [64, 4], full bias
    T1f = (W1.T + b1 / 2).astype(np.float16)      # [128, 4], half bias x2
    # t0 variants: block-diagonal two-sample gather, variant qq at cols 32qq:
    #   local col 8qq+o   <- rows 0-63:  T0f   (sample A)
    #   local col 8qq+4+o <- rows 64-127: T0f  (sample B)
    ta = np.zeros((128, 128), np.float16)
    for qq in range(4):
        ta[0:64, 32 * qq + 8 * qq:32 * qq + 8 * qq + 4] = T0f
        ta[64:128, 32 * qq + 8 * qq + 4:32 * qq + 8 * qq + 8] = T0f
    # t1 variants: variant qq at cols 32qq, local col 4qq+o <- T1f
    tb = np.zeros((128, 256), np.float16)
    for qq in range(8):
        tb[:, 32 * qq + 4 * qq:32 * qq + 4 * qq + 4] = T1f
    return ta, tb


def kernel(action_indecies, action_n_obj, action_types, W0, b0, W1, b1,
           **_unused):
    from concourse.bass_utils import run_bass_kernel_spmd

    idx = np.asarray(action_indecies, dtype=np.int32)
    typ = np.asarray(action_types, dtype=np.int32)
    B = idx.shape[0]
    b_core = B // N_CORES
    assert b_core * N_CORES == B
    b_cols = G * S

    ta, tb = _tables(W0, b0, W1, b1)
    selq = _selq()

    if b_cols not in _NC_CACHE:
        _NC_CACHE[b_cols] = _build_nc(b_cols)
    nc = _NC_CACHE[b_cols]

    perms = []
    in_maps = []
    for k in range(N_CORES):
        ik = idx[k * b_core:(k + 1) * b_core]
        tk = typ[k * b_core:(k + 1) * b_core]
        p0 = np.flatnonzero(tk == 0)
        p1 = np.flatnonzero(tk == 1)
        n0, n1 = len(p0), len(p1)
        assert n0 <= 2 * G0 * S and n1 <= G1 * S, (n0, n1)
        e0 = np.zeros(2 * G0 * S, np.int32)
        e0[:n0] = ik[p0, 0]
        v1 = np.zeros((G1 * S, 2), np.int32)
        v1[:n1] = ik[p1, :2]
        colvals = np.concatenate([e0.reshape(-1, 2), v1], axis=0)
        perms.append((p0, p1))
        in_maps.append({"idx": np.ascontiguousarray(colvals),
                        "taba": ta, "tabb": tb, "selq": selq})

    global _last_in_maps
    _last_in_maps = in_maps
    res = run_bass_kernel_spmd(nc, in_maps, core_ids=list(range(N_CORES)))

    outs = []
    for k, r in enumerate(res.results):
        p0, p1 = perms[k]
        n0, n1 = len(p0), len(p1)
        oa = r["outa"]  # [3, 4, 32, S]; band a row 8qq+4u+o, quad = b*4+qq
        ob = r["outb"]  # [3, 4, 32, S]; band a row 4qq+o,   quad = b*8+qq
        # order t0 samples: (quad, a, s, u) -> col = ((quad*4+a)*S+s)*2+u
        a6 = oa.reshape(3, 4, 4, 2, 4, S)          # [b, a, qq, u, o, s]
        a6 = np.transpose(a6, (0, 2, 1, 5, 3, 4))  # [b, qq, a, s, u, o]
        t0vals = a6.reshape(-1, 4)[:2 * G0 * S][:n0]
        b5 = ob.reshape(3, 4, 8, 4, S)             # [b, a, qq, o, s]
        b5 = np.transpose(b5, (0, 2, 1, 4, 3))     # [b, qq, a, s, o]
        t1vals = b5.reshape(-1, 4)[:G1 * S][:n1]
        o = np.empty((b_core, 4), np.float16)
        o[p0] = t0vals
        o[p1] = t1vals
        outs.append(o)
    return np.ascontiguousarray(
        np.concatenate(outs, axis=0).astype(np.float32))
